# revision 1
# baseline (speedup 1.0000x reference)
"""Trainium2 Bass kernel for GQA MultiHeadAttention (B=1, S=2048, D=4096,
H=32 query heads, HKV=8 kv heads, DK=DV=128), tensor-parallel across heads
on 8 NeuronCores.

Sharding: core c owns query heads 4c..4c+3 and kv head c (GQA group) and
computes its 512 attention features. The transposed attention output is
AllGathered across cores in four per-q-block collectives (each ~20us,
hidden behind later blocks' attention compute), then each core computes a
512-row slice of the transposed output projection. Host side:
pre-transpose/cast inputs, final concat + transpose.

Schedule highlights (all verified in TimelineSim with a device-calibrated
collective cost, and against device nrep-slope timing):
- K-proj and Q-proj are interleaved at chunk granularity: the kt/qt DMA
  streams alternate, and each K chunk's 4 matmuls are followed by one Q
  "unit" (one head-feature x one sb-pair of the previous contraction
  block) so the PE stays busy through the DMA-bound K stream. Q units
  use the pv/den PSUM slots; K owns the 4 "ps" slots.  Q's last block
  is woven into the (DMA-bound) V stream the same way.
- wk/wq are streamed in per-block slices placed in the DMA queue just
  before their consumers, never stalling the activation streams; wd
  shares the wq SBUF slot and loads during attention.
- Attention fuses score->exp->mask->den/pv per k-tile with a small
  rotating E pool.  Diagonal tiles skip the fully-masked leading
  128*d query columns in the score/exp/mask/den/pv ops (exact).
- The final q-block's out-proj runs in two dsub-pair passes so the
  first pair's writeback overlaps the second pair's matmuls; psum
  drains alternate DVE/Act.

Self-contained: hardcodes all shapes; inputs are the full unsharded tensors
keyed as in the problem's setup_inputs().
"""

import numpy as np
import ml_dtypes

import concourse.bacc as bacc
import concourse.mybir as mybir
from concourse.tile import TileContext
from concourse.bass_utils import run_bass_kernel_spmd

BF16 = mybir.dt.bfloat16
F32 = mybir.dt.float32

N_CORES = 8
S = 2048            # sequence length
D = 4096            # model dim
DK = 128            # head dim
NH_LOC = 4          # query heads per core
FLOC = NH_LOC * DK  # per-core attention features (512)
NDC = D // 128      # contraction chunks of 128 over D (32)
SB = 512            # q/s block width
NSB = S // SB       # 4
NST = S // 128      # 16 seq tiles of 128

_DMA_TYPES = ("InstDMACopy", "InstDMATranspose")


def _legalize_dma_waits(nc):
    """DMA pseudo-instructions encode at most ONE sem wait (the ISA events
    slot). If Tile's sem assignment leaves more on a DMA, walrus rejects it
    ("Too many sync wait commands"). Hoist all but the last wait onto fresh
    nop instructions inserted immediately before the DMA on the same engine —
    the sequencer executes them in order, so semantics are identical."""
    ctr = 0
    for f in nc.m.functions:
        for blk in f.blocks:
            out = []
            changed = False
            for inst in blk.instructions:
                si = inst.sync_info
                if (
                    si is not None
                    and len(si.on_wait) > 1
                    and type(inst).__name__ in _DMA_TYPES
                ):
                    waits = list(si.on_wait)
                    for w in waits[:-1]:
                        nop = mybir.InstNoOp(
                            name=f"I-dmawaitfix-{ctr}", ins=[], outs=[]
                        )
                        ctr += 1
                        nop.engine = inst.engine
                        nop.sync_info = mybir.SyncInfo(on_wait=[w], on_update=[])
                        out.append(nop)
                    inst.sync_info = mybir.SyncInfo(
                        on_wait=[waits[-1]], on_update=list(si.on_update)
                    )
                    changed = True
                out.append(inst)
            if changed:
                blk.instructions = out
    return ctr


def _build(stage=4, nrep=1):
    # stage: 1=projections only, 2=+attention (no collective), 4=full
    nc = bacc.Bacc("TRN2", target_bir_lowering=False, num_devices=N_CORES,
                   dynamic_dma_scratch_size=2048)

    # ---- I/O ----
    qT = nc.dram_tensor("qT", [D, S], BF16, kind="ExternalInput")
    kT = nc.dram_tensor("kT", [D, S], BF16, kind="ExternalInput")
    vT = nc.dram_tensor("vT", [D, S], BF16, kind="ExternalInput")
    # weights arrive host-pre-rearranged to partition-major [128, chunk*f]
    # so every device load is a plain contiguous full-rate DMA (the
    # "(c p) f -> p c f" gather pattern only reads 256B lines for DK-wide
    # weights and runs at quarter rate)
    wqT = nc.dram_tensor("wqT", [128, NDC * FLOC], BF16, kind="ExternalInput")
    wkT = nc.dram_tensor("wkT", [128, NDC * DK], BF16, kind="ExternalInput")
    wvT = nc.dram_tensor("wvT", [128, NDC * DK], BF16, kind="ExternalInput")
    wdT = nc.dram_tensor("wdT", [128, NDC * FLOC], BF16, kind="ExternalInput")
    masks = nc.dram_tensor("masks", [128, 4 * SB], BF16, kind="ExternalInput")
    ident = nc.dram_tensor("ident", [128, 128], BF16, kind="ExternalInput")
    outT = nc.dram_tensor("outT", [FLOC, S], F32, kind="ExternalOutput")

    NBLK = 8
    BLK = NDC // NBLK  # 4 chunks per Q contraction block (shorter warmup:
    # Q units need only a 4-chunk backlog before the first unit can run)

    with TileContext(nc) as tc:
        with (
            tc.tile_pool(name="consts", bufs=1) as consts,
            tc.tile_pool(name="kvw", bufs=1) as kvw,
            tc.tile_pool(name="bigw", bufs=1) as bigw,
            tc.tile_pool(name="persist", bufs=1) as persist,
            tc.tile_pool(name="qstream", bufs=8) as qstream,
            tc.tile_pool(name="kstream", bufs=2) as kstream,
            tc.tile_pool(name="vstream", bufs=8) as vstream,
            tc.tile_pool(name="erot", bufs=6) as erot,
            tc.tile_pool(name="small", bufs=1) as small,
            tc.tile_pool(name="attnout", bufs=2) as attnout,
            tc.tile_pool(name="atin", bufs=10) as atin,
            tc.tile_pool(name="osb", bufs=2) as osb,
            tc.tile_pool(name="ps", bufs=4, space="PSUM") as ps,
            tc.tile_pool(name="pspv", bufs=2, space="PSUM") as pspv,
            tc.tile_pool(name="psden", bufs=2, space="PSUM") as psden,
            tc.tile_pool(name="dram", bufs=1, space="DRAM") as dram,
        ):
            def one_rep(rep):
                # wk first: K-proj's first matmul waits only on wk + kt[0]
                wk_sb = kvw.tile([128, NDC, DK], BF16, name="wk_sb", tag="kvw")
                nc.sync.dma_start(wk_sb[:, 0:BLK, :], wkT[:, 0:BLK * DK])
                ones_sb = consts.tile([128, 128], BF16, name="ones_sb")
                nc.vector.memset(ones_sb[:], 1.0)

                # persistent activations
                QT_sb = persist.tile([128, NH_LOC, S], BF16, name="QT_sb")
                KT_sb = persist.tile([128, S], BF16, name="KT_sb")
                V_sb = persist.tile([128, NST, DK], BF16, name="V_sb")
                QTacc = persist.tile([128, NH_LOC, S], F32, name="QTacc")

                # per-q-block DRAM bounce buffers for the collectives
                attn_loc = [
                    dram.tile([FLOC, SB], BF16, name=f"attn_loc{qb}", tag=f"al{qb}")
                    for qb in range(NSB)
                ]
                attn_gath = [
                    dram.tile([N_CORES * FLOC, SB], BF16, name=f"attn_gath{qb}",
                              tag=f"ag{qb}", addr_space="Shared")
                    for qb in range(NSB)
                ]

                # ---- front: K-proj and Q-proj interleaved at chunk level ----
                # DMA order: wk, kt0, qt0, wq, [kt_i, qt_i]..., wv, ident,
                # masks, vt stream. The PE alternates 4 K matmuls per chunk
                # with one Q "unit" (one head-feature x one sb-pair of the
                # PREVIOUS contraction block) so the PE stays busy through the
                # DMA-bound K stream. Q units use the pv/den PSUM slots
                # (2 apiece, alternating), K owns the 4 "ps" slots.
                q_chunks = {}
                wq_holder = []

                def emit_q_unit(b, f, pair):
                    tpool, ttag = (pspv, "pv") if pair == 0 else (psden, "den")
                    qp = [
                        tpool.tile([128, SB], F32, name=f"qp{j}", tag=ttag)
                        for j in range(2)
                    ]
                    for ch in range(BLK):
                        dc = BLK * b + ch
                        for j in range(2):
                            s2 = 2 * pair + j
                            nc.tensor.matmul(
                                qp[j][:],
                                lhsT=wq_holder[0][:, dc, f * 128:(f + 1) * 128],
                                rhs=q_chunks[dc][:, s2 * SB:(s2 + 1) * SB],
                                start=(ch == 0),
                                stop=(ch == BLK - 1),
                            )
                    for j in range(2):
                        s2 = 2 * pair + j
                        dst_acc = QTacc[:, f, s2 * SB:(s2 + 1) * SB]
                        if b == 0:
                            nc.vector.tensor_copy(dst_acc, qp[j][:])
                        elif b < NBLK - 1:
                            nc.vector.tensor_tensor(
                                dst_acc, dst_acc, qp[j][:], mybir.AluOpType.add
                            )
                        else:
                            nc.vector.tensor_tensor(
                                QT_sb[:, f, s2 * SB:(s2 + 1) * SB],
                                dst_acc, qp[j][:], mybir.AluOpType.add,
                            )

                k_ps = [ps.tile([128, SB], F32, name=f"kps{i}", tag="ps")
                        for i in range(NSB)]
                for b in range(NBLK):
                    for i in range(BLK):
                        dc = BLK * b + i
                        kt_c = kstream.tile([128, S], BF16, name="kt_c", tag="kt")
                        nc.sync.dma_start(kt_c[:], kT[dc * 128:(dc + 1) * 128, :])
                        qt_c = qstream.tile([128, S], BF16, name="qt_c", tag="qt")
                        nc.sync.dma_start(qt_c[:], qT[dc * 128:(dc + 1) * 128, :])
                        q_chunks[dc] = qt_c
                        if dc == 1:
                            wq_sb = bigw.tile([128, NDC, FLOC], BF16,
                                              name="wq_sb", tag="bigw")
                            wq_holder.append(wq_sb)
                        if dc % BLK == 0 and 0 < dc:
                            # next wk slice (256KB) just ahead of its block
                            nc.sync.dma_start(
                                wk_sb[:, dc:dc + BLK, :],
                                wkT[:, dc * DK:(dc + BLK) * DK])
                        if dc % BLK == 1:
                            # stream wq in per-block 1MB slices woven between
                            # kt/qt pairs: slice j (dc 8j..8j+7) is needed only
                            # by block j's Q units, which run during block j+1,
                            # so each 2.9us slice hides in a PE-bound window
                            # instead of stalling the whole kt stream at once.
                            j = dc // BLK
                            nc.sync.dma_start(
                                wq_holder[0][:, BLK * j:BLK * (j + 1), :],
                                wqT[:, BLK * FLOC * j:BLK * FLOC * (j + 1)])
                        for sb in range(NSB):
                            nc.tensor.matmul(
                                k_ps[sb][:],
                                lhsT=wk_sb[:, dc, :],
                                rhs=kt_c[:, sb * SB:(sb + 1) * SB],
                                start=(dc == 0),
                                stop=(dc == NDC - 1),
                            )
                        if b >= 1:
                            emit_q_unit(b - 1, i, 0)
                            emit_q_unit(b - 1, i, 1)
                for sb in range(NSB):
                    nc.vector.tensor_copy(KT_sb[:, sb * SB:(sb + 1) * SB], k_ps[sb][:])

                # ---- V projection (DMA-bound) with Q's last block woven in ----
                # only wv's first slice goes ahead of vt0; the rest of wv and
                # the ident/masks constants are woven into the vt stream so
                # the DMA-critical vt chunks are not pushed back
                wv_sb = kvw.tile([128, NDC, DK], BF16, name="wv_sb", tag="kvw")
                nc.sync.dma_start(wv_sb[:, 0:BLK, :], wvT[:, 0:BLK * DK])
                ident_sb = consts.tile([128, 128], BF16, name="ident_sb")
                masks_sb = consts.tile([128, 4 * SB], BF16, name="masks_sb")

                VT_sb = persist.tile([128, S], BF16, name="VT_sb")
                v_ps = [ps.tile([128, SB], F32, name=f"vps{i}", tag="ps")
                        for i in range(NSB)]
                for dc in range(NDC):
                    if dc % BLK == 2 and dc < NDC - BLK:
                        j = dc // BLK + 1
                        nc.sync.dma_start(
                            wv_sb[:, BLK * j:BLK * (j + 1), :],
                            wvT[:, BLK * DK * j:BLK * DK * (j + 1)])
                    if dc == 5:
                        nc.sync.dma_start(ident_sb[:], ident[:])
                        nc.sync.dma_start(masks_sb[:], masks[:])
                    vt_c = vstream.tile([128, S], BF16, name="vt_c", tag="vt")
                    nc.sync.dma_start(vt_c[:], vT[dc * 128:(dc + 1) * 128, :])
                    for sb in range(NSB):
                        nc.tensor.matmul(
                            v_ps[sb][:],
                            lhsT=wv_sb[:, dc, :],
                            rhs=vt_c[:, sb * SB:(sb + 1) * SB],
                            start=(dc == 0),
                            stop=(dc == NDC - 1),
                        )
                    if dc % 4 == 3:
                        u = dc // 4
                        emit_q_unit(NBLK - 1, u // 2, u % 2)
                        # NBLK=8: same 8 units, just from the 4-chunk last block
                for sb in range(NSB):
                    nc.vector.tensor_copy(VT_sb[:, sb * SB:(sb + 1) * SB], v_ps[sb][:])
                for g in range(NST // 4):
                    tp = ps.tile([128, 4, 128], BF16, name="tp", tag="ps")
                    for j in range(4):
                        st = 4 * g + j
                        nc.tensor.transpose(
                            tp[:, j, :], VT_sb[:, st * 128:(st + 1) * 128],
                            ident_sb[:])
                    # one wide copy per 4 transposes: the phase runs at PE
                    # pace instead of per-tile DVE-copy pace
                    nc.vector.tensor_copy(V_sb[:, 4 * g:4 * (g + 1), :], tp[:])

                if stage == 1:
                    for hh in range(NH_LOC):
                        for sb in range(NSB):
                            o_sb = osb.tile([128, SB], F32, name="o_sb", tag="osb")
                            nc.vector.tensor_copy(o_sb[:], QT_sb[:, hh, sb * SB:(sb + 1) * SB])
                            nc.sync.dma_start(
                                outT[hh * 128:(hh + 1) * 128, sb * SB:(sb + 1) * SB], o_sb[:]
                            )

                # ---- attention, fused score/exp/den/pv per k-tile so E tiles
                # ---- rotate in a small pool; q-block outer so each block's
                # ---- AllGather overlaps later blocks' compute
                if stage >= 2:
                    for qb in range(NSB):
                        nkt = 4 * qb + 4  # causal: k-tiles 0..4qb+3
                        for h in range(NH_LOC):
                            den_ps = psden.tile([128, SB], F32, name="den_ps", tag="den")
                            att_ps = pspv.tile([128, SB], F32, name="att_ps", tag="pv")
                            split = nkt <= 4  # small blocks: issue all scores
                            # first so the exp latency amortizes across the head
                            E_tiles = []

                            def score_exp(kt):
                                # diagonal tile d: query cols < 128*d are fully
                                # below the causal mask -> skip them in the
                                # score matmul, exp, mask, den and pv.
                                d = kt - 4 * qb
                                off = 128 * d if d >= 1 else 0
                                st_ps = ps.tile([128, SB], F32, name="st_ps", tag="ps")
                                nc.tensor.matmul(
                                    st_ps[:, off:],
                                    lhsT=KT_sb[:, kt * 128:(kt + 1) * 128],
                                    rhs=QT_sb[:, h, qb * SB + off:(qb + 1) * SB],
                                    start=True,
                                    stop=True,
                                )
                                E1 = erot.tile([128, SB], BF16, name="E1", tag="E")
                                nc.scalar.activation(
                                    E1[:, off:], st_ps[:, off:],
                                    mybir.ActivationFunctionType.Exp
                                )
                                if d >= 0:  # diagonal tile -> causal mask
                                    nc.vector.tensor_tensor(
                                        E1[:, off:],
                                        E1[:, off:],
                                        masks_sb[:, d * SB + off:(d + 1) * SB],
                                        mybir.AluOpType.mult,
                                    )
                                return E1

                            def den_pv(kt, E1):
                                d = kt - 4 * qb
                                off = 128 * d if d >= 1 else 0
                                nc.tensor.matmul(
                                    den_ps[:, off:],
                                    lhsT=ones_sb[:, :],
                                    rhs=E1[:, off:],
                                    start=(kt == 0),
                                    stop=(kt == nkt - 1),
                                )
                                nc.tensor.matmul(
                                    att_ps[:, off:],
                                    lhsT=V_sb[:, kt, :],
                                    rhs=E1[:, off:],
                                    start=(kt == 0),
                                    stop=(kt == nkt - 1),
                                )

                            if split:
                                for kt in range(nkt):
                                    E_tiles.append(score_exp(kt))
                                for kt in range(nkt):
                                    den_pv(kt, E_tiles[kt])
                            else:
                                for kt in range(nkt):
                                    den_pv(kt, score_exp(kt))
                            # normalize: attn[dv, q] /= den[q]. den_ps was computed
                            # with a full ones-matrix lhsT, so every PSUM partition
                            # holds the denominator row -> plain elementwise multiply.
                            rec = small.tile([128, SB], F32, name="rec", tag="rec")
                            nc.vector.reciprocal(rec[:], den_ps[:])
                            attn_t = attnout.tile([128, SB], BF16, name="attn_t", tag="attn")
                            nc.vector.tensor_tensor(
                                attn_t[:], att_ps[:], rec[:], mybir.AluOpType.mult
                            )
                            nc.sync.dma_start(
                                attn_loc[qb][h * 128:(h + 1) * 128, :], attn_t[:]
                            )
                            if stage == 2:
                                o_sb = osb.tile([128, SB], F32, name="o_sb", tag="osb2")
                                nc.vector.tensor_copy(o_sb[:], attn_t[:])
                                nc.sync.dma_start(
                                    outT[h * 128:(h + 1) * 128, qb * SB:(qb + 1) * SB],
                                    o_sb[:],
                                )

                        if stage >= 2:
                            # wd loads during attention in per-q-block 1MB
                            # slices (a single 4MB load here delays this
                            # q-block's attn writeback and so its gather)
                            if qb == 0:
                                wd_sb = bigw.tile([128, NDC, FLOC], BF16,
                                                  name="wd_sb", tag="bigw")
                            wdc = NDC // NSB  # 8 chunks per q-block slice
                            nc.sync.dma_start(
                                wd_sb[:, wdc * qb:wdc * (qb + 1), :],
                                wdT[:, wdc * FLOC * qb:wdc * FLOC * (qb + 1)])

                        if stage >= 4:
                            # gather this q-block's attn^T from all cores
                            nc.gpsimd.collective_compute(
                                "AllGather",
                                mybir.AluOpType.bypass,
                                replica_groups=[list(range(N_CORES))],
                                ins=[attn_loc[qb][:]],
                                outs=[attn_gath[qb][:]],
                            )

                # ---- output projection per q-block: outT[d, qb] (512x512 slice) ----
                if stage >= 4:
                    def op_block(qb, dsubs):
                        o_ps = [
                            ps.tile([128, SB], F32, name=f"ops{d2}", tag="ps")
                            if d2 < 2 else
                            (pspv.tile([128, SB], F32, name=f"ops{d2}", tag="pv")
                             if d2 == 2 else
                             psden.tile([128, SB], F32, name=f"ops{d2}", tag="den"))
                            for d2 in dsubs
                        ]
                        for fc2 in range(NDC // 2):
                            at_c = atin.tile([128, 2, SB], BF16, name="at_c",
                                             tag="atin")
                            nc.sync.dma_start(
                                at_c[:],
                                attn_gath[qb][fc2 * 256:(fc2 + 1) * 256, :]
                                .rearrange("(j p) q -> p j q", p=128),
                            )
                            for j2 in range(2):
                                fc = 2 * fc2 + j2
                                for j, dsub in enumerate(dsubs):
                                    nc.tensor.matmul(
                                        o_ps[j][:],
                                        lhsT=wd_sb[:, fc, dsub * 128:(dsub + 1) * 128],
                                        rhs=at_c[:, j2, :],
                                        start=(fc == 0),
                                        stop=(fc == NDC - 1),
                                    )
                        for j, dsub in enumerate(dsubs):
                            o_sb = osb.tile([128, SB], F32, name="o_sb", tag="osb")
                            if j % 2 == 0:
                                nc.vector.tensor_copy(o_sb[:], o_ps[j][:])
                            else:
                                nc.scalar.copy(o_sb[:], o_ps[j][:])
                            nc.sync.dma_start(
                                outT[dsub * 128:(dsub + 1) * 128,
                                     qb * SB:(qb + 1) * SB],
                                o_sb[:],
                            )

                    for qb in range(NSB - 1):
                        op_block(qb, [0, 1, 2, 3])
                    # last q-block in two dsub-pair passes: the first pair's
                    # writeback overlaps the second pair's matmuls
                    op_block(NSB - 1, [0, 1])
                    op_block(NSB - 1, [2, 3])

            for rep in range(nrep):
                one_rep(rep)

    nc.compile()
    _legalize_dma_waits(nc)
    nc.codegen_inst_isa_subclasses()
    return nc


_NC_CACHE = None


def _get_nc():
    global _NC_CACHE
    if _NC_CACHE is None:
        _NC_CACHE = _build()
    return _NC_CACHE


def _make_in_maps(q, k, v, Wq, Wk, Wv, Wd):
    bf = ml_dtypes.bfloat16
    scale = np.float32(DK) ** -0.5
    qT = np.ascontiguousarray(q.reshape(S, D).T).astype(bf)
    kT = np.ascontiguousarray(k.reshape(S, D).T).astype(bf)
    vT = np.ascontiguousarray(v.reshape(S, D).T).astype(bf)

    kp = np.arange(128, dtype=np.int32)[:, None]
    qf = np.arange(SB, dtype=np.int32)[None, :]
    masks = np.concatenate(
        [(qf >= kp + 128 * d).astype(np.float32) for d in range(4)], axis=1
    ).astype(bf)
    ident = np.eye(128, dtype=np.float32).astype(bf)

    def pmajor(wT):
        # [D, f] -> partition-major [128, NDC * f]: row p holds chunk-major
        # per-chunk rows so device loads are plain contiguous DMAs
        f = wT.shape[1]
        return np.ascontiguousarray(
            wT.reshape(NDC, 128, f).transpose(1, 0, 2).reshape(128, NDC * f)
        )

    in_maps = []
    for c in range(N_CORES):
        fs = slice(FLOC * c, FLOC * (c + 1))
        ks = slice(DK * c, DK * (c + 1))
        in_maps.append({
            "qT": qT,
            "kT": kT,
            "vT": vT,
            "wqT": pmajor((Wq[fs, :] * scale).T).astype(bf),
            "wkT": pmajor(Wk[ks, :].T).astype(bf),
            "wvT": pmajor(Wv[ks, :].T).astype(bf),
            "wdT": pmajor(Wd[fs, :].T).astype(bf),
            "masks": masks,
            "ident": ident,
        })
    return in_maps


def _assemble(results):
    outT_full = np.concatenate([r["outT"] for r in results], axis=0)  # [4096, 2048]
    return np.ascontiguousarray(outT_full.T).reshape(1, S, D).astype(np.float32)


def kernel(q, k, v, Wq, Wk, Wv, Wd, _trace=False, **_ignored):
    nc = _get_nc()
    in_maps = _make_in_maps(
        np.asarray(q, np.float32), np.asarray(k, np.float32),
        np.asarray(v, np.float32), np.asarray(Wq, np.float32),
        np.asarray(Wk, np.float32), np.asarray(Wv, np.float32),
        np.asarray(Wd, np.float32),
    )
    res = run_bass_kernel_spmd(
        nc, in_maps, core_ids=list(range(N_CORES)), trace=_trace
    )
    out = _assemble(res.results)
    if _trace:
        return out, res
    return out

